# revision 2
# baseline (speedup 1.0000x reference)
"""Trainium2 Bass kernel for a dense transformer block.

Block: LN1 -> 8-head self-attention -> residual -> LN2 -> 4x-MLP(gelu) -> residual
Input hidden_states [2, 4096, 512] fp32.

Sharding: data-parallel over query tokens. 8192 tokens / 8 cores = 1024
queries per core (core c: batch c//4, quarter c%4). Each core redundantly
computes LN1 + K/V over its batch's full 4096 tokens (no collectives), then
attention for its own 1024 queries, O-proj, LN2, MLP.

Host-side input prep (exact math, no approximation beyond bf16 matmul cast):
  - the per-core token axis is ROTATED so own queries are always rows 0-1023
    (softmax over keys is permutation invariant, so K/V order is irrelevant)
  - LN gammas/betas folded into the consuming weight matrices:
      Wq' = diag(g1) Wq, bq' = bq + be1 @ Wq   (same for Wk, Wv, W1)
  - V bias folded past attention: bo' = bo + bv @ Wo
  - weights cast to bf16 (matmuls run bf16 with fp32 PSUM accumulation)

On-chip layout: activations are kept feature-major ("T" = transposed,
[feature -> partitions, token -> free]) for all matmul contractions; LN and
residuals run token-major; the two layouts are bridged with bf16 DMA
transposes through DRAM scratch.

Attention per head: S^T[k,q] = KT.lhsT @ QT.rhs (64-deep contraction,
row-tiled 2 heads per 512-cycle window), exp on ScalarE (scale=1/8 folded
in; no max subtraction -- logits here are bounded ~ +-7 so fp32 exp is
safe), then ctx^T = [V|1].lhsT @ expS^T accumulated over k chunks; the ones
column yields the softmax denominator in PSUM row 64 for free.
"""

import sys

if "/opt/trn_rl_repo" not in sys.path:
    sys.path.insert(0, "/opt/trn_rl_repo")

import numpy as np
import ml_dtypes

import concourse.bass as bass
import concourse.tile as tile
from concourse import bacc, mybir

F32 = mybir.dt.float32
BF16 = mybir.dt.bfloat16
AF = mybir.ActivationFunctionType
OP = mybir.AluOpType

B, S, H = 2, 4096, 512
NH, HD = 8, 64
F = 4 * H  # 2048
N_CORES = 8
TQ = (B * S) // N_CORES  # 1024 own queries per core
EPS = 1e-5

DC = H // 128     # 4   d chunks
HC = H // 128     # 4   head-pair chunks (2 heads each)
FC = F // 128     # 16  mlp hidden chunks
KC = S // 128     # 32  key chunks
QW = 512          # matmul free-dim window
NQ = TQ // QW     # 2   query windows per core
NT = S // QW      # 8   token windows (full batch)


def _ln_tile(nc, pool, x_tile, out_tile, eps_tile):
    """LayerNorm (pure normalize) of one [128, 512] token-major tile."""
    stats = pool.tile([128, 6], F32, tag="ln_stats")
    nc.vector.bn_stats(out=stats[:], in_=x_tile)
    mv = pool.tile([128, 2], F32, tag="ln_mv")
    nc.vector.bn_aggr(out=mv[:], in_=stats[:])
    std = pool.tile([128, 1], F32, tag="ln_std")
    nc.scalar.activation(out=std[:], in_=mv[:, 1:2], func=AF.Sqrt, bias=eps_tile[:])
    rstd = pool.tile([128, 1], F32, tag="ln_rstd")
    nc.vector.reciprocal(out=rstd[:], in_=std[:])
    nc.vector.tensor_scalar(
        out=out_tile,
        in0=x_tile,
        scalar1=mv[:, 0:1],
        scalar2=rstd[:],
        op0=OP.subtract,
        op1=OP.mult,
    )


def build():
    nc = bacc.Bacc(trn_type="TRN2", target_bir_lowering=False)

    # ---- I/O -------------------------------------------------------------
    x_d = nc.dram_tensor("x", [S, H], F32, kind="ExternalInput")
    wq_d = nc.dram_tensor("wq", [128, DC, H], BF16, kind="ExternalInput")
    wk_d = nc.dram_tensor("wk", [128, DC, H], BF16, kind="ExternalInput")
    wv_d = nc.dram_tensor("wv", [128, DC, H], BF16, kind="ExternalInput")
    wo_d = nc.dram_tensor("wo", [128, HC, H], BF16, kind="ExternalInput")
    w1_d = nc.dram_tensor("w1", [128, DC, F], BF16, kind="ExternalInput")
    w2_d = nc.dram_tensor("w2", [128, FC, H], BF16, kind="ExternalInput")
    bq_d = nc.dram_tensor("bq", [128, DC], F32, kind="ExternalInput")
    bk_d = nc.dram_tensor("bk", [128, DC], F32, kind="ExternalInput")
    bo_d = nc.dram_tensor("bo", [128, DC], F32, kind="ExternalInput")
    b1_d = nc.dram_tensor("b1", [128, FC], F32, kind="ExternalInput")
    b2_d = nc.dram_tensor("b2", [128, DC], F32, kind="ExternalInput")
    out_d = nc.dram_tensor("out", [TQ, H], F32, kind="ExternalOutput")

    # DRAM scratch for bf16 transposes
    xn_d = nc.dram_tensor("xn_scratch", [S, H], BF16)
    ao_d = nc.dram_tensor("ao_scratch", [H, TQ], BF16)
    yn_d = nc.dram_tensor("yn_scratch", [TQ, H], BF16)
    z_d = nc.dram_tensor("z_scratch", [H, TQ], BF16)

    x_t = x_d[:, :].rearrange("(n p) d -> n p d", p=128)       # [32,128,512]
    xn_t = xn_d[:, :].rearrange("(n p) d -> n p d", p=128)
    yn_t = yn_d[:, :].rearrange("(n p) d -> n p d", p=128)
    out_t = out_d[:, :].rearrange("(n p) d -> n p d", p=128)   # [8,128,512]
    ao_t = ao_d[:, :].rearrange("(c p) q -> p c q", p=128)     # [128,4,1024]
    z_t = z_d[:, :].rearrange("(c p) q -> p c q", p=128)

    with tile.TileContext(nc) as tc:
        with (
            tc.tile_pool(name="persist", bufs=1) as P,
            tc.tile_pool(name="temps", bufs=3) as T,
            tc.tile_pool(name="small", bufs=4) as SM,
        ):
            eps_tile = P.tile([128, 1], F32, tag="eps")
            nc.vector.memset(eps_tile, EPS)
            ones_sb = P.tile([1, 64], BF16, tag="ones")
            nc.vector.memset(ones_sb, 1.0)

            # biases
            bq_sb = P.tile([128, DC], F32, tag="bq")
            nc.sync.dma_start(out=bq_sb[:], in_=bq_d[:, :])
            bk_sb = P.tile([128, DC], F32, tag="bk")
            nc.sync.dma_start(out=bk_sb[:], in_=bk_d[:, :])
            bo_sb = P.tile([128, DC], F32, tag="bo")
            nc.sync.dma_start(out=bo_sb[:], in_=bo_d[:, :])
            b1_sb = P.tile([128, FC], F32, tag="b1")
            nc.sync.dma_start(out=b1_sb[:], in_=b1_d[:, :])
            b2_sb = P.tile([128, DC], F32, tag="b2")
            nc.sync.dma_start(out=b2_sb[:], in_=b2_d[:, :])

            # persistent activations
            xnT = P.tile([128, DC, S], BF16, tag="big32")       # LN1(x)^T
            KT = P.tile([128, HC, S], BF16, tag="slotA")        # K^T
            Vp = P.tile([128, KC, NH, 65], BF16, tag="slotB")   # V | ones
            QT = P.tile([128, HC, TQ], BF16, tag="slotE")       # Q^T
            ctxT = P.tile([128, HC, TQ], BF16, tag="ctxT")
            h_sb = P.tile([128, TQ // 128, H], F32, tag="h")    # residual 1
            wqkv = P.tile([128, 3, DC, H], BF16, tag="slotC")
            wo_sb = P.tile([128, HC, H], BF16, tag="wo")

            nc.sync.dma_start(out=wqkv[:, 0], in_=wq_d[:, :, :])
            nc.sync.dma_start(out=wqkv[:, 1], in_=wk_d[:, :, :])
            nc.sync.dma_start(out=wqkv[:, 2], in_=wv_d[:, :, :])
            nc.sync.dma_start(out=wo_sb[:], in_=wo_d[:, :, :])

            nc.vector.memset(Vp[:, :, :, 64:65], 1.0)

            # ============ Phase 1: LN1 over all 4096 tokens ===============
            for i in range(KC):
                xt = T.tile([128, H], F32, tag="x_in")
                nc.sync.dma_start(out=xt[:], in_=x_t[i, :, :])
                xn = T.tile([128, H], BF16, tag="xn_out")
                _ln_tile(nc, SM, xt[:], xn[:], eps_tile)
                nc.sync.dma_start(out=xn_t[i, :, :], in_=xn[:])
            # transpose -> xnT
            for c in range(DC):
                nc.sync.dma_start_transpose(xnT[:, c, :], xn_d[:, c * 128:(c + 1) * 128])

            # ============ Phase 2: projections ============================
            with tc.tile_pool(name="psA", bufs=4, space="PSUM") as PSA:
                # Q^T [512, 1024] feature-major
                for hc in range(HC):
                    for q in range(NQ):
                        ps = PSA.tile([128, QW], F32, tag="proj")
                        for dc in range(DC):
                            nc.tensor.matmul(
                                out=ps[:],
                                lhsT=wqkv[:, 0, dc, hc * 128:(hc + 1) * 128],
                                rhs=xnT[:, dc, q * QW:(q + 1) * QW],
                                start=(dc == 0),
                                stop=(dc == DC - 1),
                            )
                        nc.vector.tensor_scalar_add(
                            out=QT[:, hc, q * QW:(q + 1) * QW],
                            in0=ps[:],
                            scalar1=bq_sb[:, hc:hc + 1],
                        )
                # K^T [512, 4096]
                for hc in range(HC):
                    for t in range(NT):
                        ps = PSA.tile([128, QW], F32, tag="proj")
                        for dc in range(DC):
                            nc.tensor.matmul(
                                out=ps[:],
                                lhsT=wqkv[:, 1, dc, hc * 128:(hc + 1) * 128],
                                rhs=xnT[:, dc, t * QW:(t + 1) * QW],
                                start=(dc == 0),
                                stop=(dc == DC - 1),
                            )
                        nc.vector.tensor_scalar_add(
                            out=KT[:, hc, t * QW:(t + 1) * QW],
                            in0=ps[:],
                            scalar1=bk_sb[:, hc:hc + 1],
                        )
                # V token-major, head-interleaved with ones column
                for kc in range(KC):
                    ps = PSA.tile([128, QW], F32, tag="proj")
                    for dc in range(DC):
                        nc.tensor.matmul(
                            out=ps[:],
                            lhsT=xnT[:, dc, kc * 128:(kc + 1) * 128],
                            rhs=wqkv[:, 2, dc, :],
                            start=(dc == 0),
                            stop=(dc == DC - 1),
                        )
                    nc.vector.tensor_copy(
                        out=Vp[:, kc, :, 0:64],
                        in_=ps[:].rearrange("p (h d) -> p h d", h=NH),
                    )

            # ============ Phase 3: attention ==============================
            with (
                tc.tile_pool(name="psS", bufs=2, space="PSUM") as PSS,
                tc.tile_pool(name="psC", bufs=1, space="PSUM") as PSC,
                tc.tile_pool(name="esb", bufs=4) as ESB,
            ):
                for p in range(HC):          # head pair
                    for q in range(NQ):      # query window
                        ctxA = PSC.tile([65, QW], F32, tag="ctxA")
                        ctxB = PSC.tile([65, QW], F32, tag="ctxB")
                        for kc in range(KC):
                            sA = PSS.tile([128, QW], F32, tag="sA")
                            sB = PSS.tile([128, QW], F32, tag="sB")
                            nc.tensor.matmul(
                                out=sA[:],
                                lhsT=KT[0:64, p, kc * 128:(kc + 1) * 128],
                                rhs=QT[0:64, p, q * QW:(q + 1) * QW],
                                start=True, stop=True,
                            )
                            nc.tensor.matmul(
                                out=sB[:],
                                lhsT=KT[64:128, p, kc * 128:(kc + 1) * 128],
                                rhs=QT[64:128, p, q * QW:(q + 1) * QW],
                                start=True, stop=True,
                            )
                            eA = ESB.tile([128, QW], BF16, tag="eA")
                            nc.scalar.activation(out=eA[:], in_=sA[:], func=AF.Exp, scale=0.125)
                            eB = ESB.tile([128, QW], BF16, tag="eB")
                            nc.scalar.activation(out=eB[:], in_=sB[:], func=AF.Exp, scale=0.125)
                            nc.tensor.matmul(
                                out=ctxA[:], lhsT=Vp[:, kc, 2 * p, :], rhs=eA[:],
                                start=(kc == 0), stop=(kc == KC - 1),
                            )
                            nc.tensor.matmul(
                                out=ctxB[:], lhsT=Vp[:, kc, 2 * p + 1, :], rhs=eB[:],
                                start=(kc == 0), stop=(kc == KC - 1),
                            )
                        # softmax divide: recip of denominators (PSUM row 64),
                        # broadcast across 64 partitions via K=1 matmul
                        rA = SM.tile([1, QW], BF16, tag="rA")
                        rB = SM.tile([1, QW], BF16, tag="rB")
                        with nc.allow_low_precision(reason="bf16 softmax denom"):
                            nc.vector.reciprocal(out=rA[:], in_=ctxA[64:65, :])
                            nc.vector.reciprocal(out=rB[:], in_=ctxB[64:65, :])
                        bc = PSS.tile([128, QW], F32, tag="bc")
                        nc.tensor.matmul(out=bc[0:64, :], lhsT=ones_sb[:], rhs=rA[:],
                                         start=True, stop=True)
                        nc.tensor.matmul(out=bc[64:128, :], lhsT=ones_sb[:], rhs=rB[:],
                                         start=True, stop=True)
                        bcs = ESB.tile([128, QW], F32, tag="bcs")
                        nc.vector.tensor_copy(out=bcs[:], in_=bc[:])
                        nc.vector.tensor_mul(
                            out=ctxT[0:64, p, q * QW:(q + 1) * QW],
                            in0=ctxA[0:64, :],
                            in1=bcs[0:64, :],
                        )
                        nc.vector.tensor_mul(
                            out=ctxT[64:128, p, q * QW:(q + 1) * QW],
                            in0=ctxB[0:64, :],
                            in1=bcs[64:128, :],
                        )

            # ============ Phase 4: O-proj, residual, LN2 ==================
            attn_oT = P.tile([128, DC, TQ], BF16, tag="slotC")
            with tc.tile_pool(name="psB", bufs=4, space="PSUM") as PSB:
                for oc in range(DC):
                    for q in range(NQ):
                        ps = PSB.tile([128, QW], F32, tag="proj")
                        for hc in range(HC):
                            nc.tensor.matmul(
                                out=ps[:],
                                lhsT=wo_sb[:, hc, oc * 128:(oc + 1) * 128],
                                rhs=ctxT[:, hc, q * QW:(q + 1) * QW],
                                start=(hc == 0),
                                stop=(hc == HC - 1),
                            )
                        nc.vector.tensor_scalar_add(
                            out=attn_oT[:, oc, q * QW:(q + 1) * QW],
                            in0=ps[:],
                            scalar1=bo_sb[:, oc:oc + 1],
                        )
            nc.sync.dma_start(out=ao_t[:, :, :], in_=attn_oT[:])
            # transpose attn_out to token-major; residual; LN2; re-transpose
            atok = P.tile([128, TQ // 128, H], BF16, tag="slotD")
            for i in range(TQ // 128):
                nc.sync.dma_start_transpose(atok[:, i, :], ao_d[:, i * 128:(i + 1) * 128])
            for i in range(TQ // 128):
                xt = T.tile([128, H], F32, tag="x_in")
                nc.sync.dma_start(out=xt[:], in_=x_t[i, :, :])
                nc.vector.tensor_add(out=h_sb[:, i, :], in0=xt[:], in1=atok[:, i, :])
                ynt = T.tile([128, H], BF16, tag="xn_out")
                _ln_tile(nc, SM, h_sb[:, i, :], ynt[:], eps_tile)
                nc.sync.dma_start(out=yn_t[i, :, :], in_=ynt[:])
            ynT = P.tile([128, DC, TQ], BF16, tag="slotC")
            for c in range(DC):
                nc.sync.dma_start_transpose(ynT[:, c, :], yn_d[:, c * 128:(c + 1) * 128])

            # ============ Phase 5: MLP ====================================
            w1_sb = P.tile([128, DC, F], BF16, tag="slotA")
            nc.sync.dma_start(out=w1_sb[:], in_=w1_d[:, :, :])
            w2_sb = P.tile([128, FC, H], BF16, tag="slotB")
            nc.sync.dma_start(out=w2_sb[:], in_=w2_d[:, :, :])
            GT = P.tile([128, FC, TQ], BF16, tag="big32")
            zT = P.tile([128, DC, TQ], BF16, tag="slotE")
            with tc.tile_pool(name="psM", bufs=4, space="PSUM") as PSM:
                for fc in range(FC):
                    for q in range(NQ):
                        ps = PSM.tile([128, QW], F32, tag="proj")
                        for dc in range(DC):
                            nc.tensor.matmul(
                                out=ps[:],
                                lhsT=w1_sb[:, dc, fc * 128:(fc + 1) * 128],
                                rhs=ynT[:, dc, q * QW:(q + 1) * QW],
                                start=(dc == 0),
                                stop=(dc == DC - 1),
                            )
                        nc.scalar.activation(
                            out=GT[:, fc, q * QW:(q + 1) * QW],
                            in_=ps[:],
                            func=AF.Gelu,
                            bias=b1_sb[:, fc:fc + 1],
                        )
                for oc in range(DC):
                    for q in range(NQ):
                        ps = PSM.tile([128, QW], F32, tag="proj")
                        for fc in range(FC):
                            nc.tensor.matmul(
                                out=ps[:],
                                lhsT=w2_sb[:, fc, oc * 128:(oc + 1) * 128],
                                rhs=GT[:, fc, q * QW:(q + 1) * QW],
                                start=(fc == 0),
                                stop=(fc == FC - 1),
                            )
                        nc.vector.tensor_scalar_add(
                            out=zT[:, oc, q * QW:(q + 1) * QW],
                            in0=ps[:],
                            scalar1=b2_sb[:, oc:oc + 1],
                        )
            nc.sync.dma_start(out=z_t[:, :, :], in_=zT[:])
            ztok = P.tile([128, TQ // 128, H], BF16, tag="slotD")
            for i in range(TQ // 128):
                nc.sync.dma_start_transpose(ztok[:, i, :], z_d[:, i * 128:(i + 1) * 128])
            for i in range(TQ // 128):
                res = T.tile([128, H], F32, tag="res")
                nc.vector.tensor_add(out=res[:], in0=h_sb[:, i, :], in1=ztok[:, i, :])
                nc.sync.dma_start(out=out_t[i, :, :], in_=res[:])

    nc.finalize()
    return nc


_NC = None


def _get_nc():
    global _NC
    if _NC is None:
        _NC = build()
    return _NC


def _tile_w(w):
    """[H?, O] fp32 -> [128, H//128, O] bf16 (contraction dim onto partitions)."""
    k, o = w.shape
    return np.ascontiguousarray(
        w.reshape(k // 128, 128, o).transpose(1, 0, 2)
    ).astype(ml_dtypes.bfloat16)


def _tile_b(b):
    """[O] fp32 -> [128, O//128] (per-partition bias per 128-chunk)."""
    return np.ascontiguousarray(b.reshape(-1, 128).T).astype(np.float32)


def prep_inmaps(inputs):
    hs = np.asarray(inputs["hidden_states"], dtype=np.float32)
    Wq = np.asarray(inputs["Wq"], dtype=np.float32)
    Wk = np.asarray(inputs["Wk"], dtype=np.float32)
    Wv = np.asarray(inputs["Wv"], dtype=np.float32)
    Wo = np.asarray(inputs["Wo"], dtype=np.float32)
    W1 = np.asarray(inputs["W1"], dtype=np.float32)
    W2 = np.asarray(inputs["W2"], dtype=np.float32)
    bq = np.asarray(inputs["bq"], dtype=np.float32)
    bk = np.asarray(inputs["bk"], dtype=np.float32)
    bv = np.asarray(inputs["bv"], dtype=np.float32)
    bo = np.asarray(inputs["bo"], dtype=np.float32)
    b1 = np.asarray(inputs["b1"], dtype=np.float32)
    b2 = np.asarray(inputs["b2"], dtype=np.float32)
    g1 = np.asarray(inputs["g1"], dtype=np.float32)
    be1 = np.asarray(inputs["be1"], dtype=np.float32)
    g2 = np.asarray(inputs["g2"], dtype=np.float32)
    be2 = np.asarray(inputs["be2"], dtype=np.float32)

    # fold LN affine params / v-bias (exact)
    wq = _tile_w(g1[:, None] * Wq)
    wk = _tile_w(g1[:, None] * Wk)
    wv = _tile_w(g1[:, None] * Wv)
    wo = _tile_w(Wo)
    w1 = _tile_w(g2[:, None] * W1)
    w2 = _tile_w(W2)
    bq_e = _tile_b(bq + be1 @ Wq)
    bk_e = _tile_b(bk + be1 @ Wk)
    bo_e = _tile_b(bo + (bv + be1 @ Wv) @ Wo)
    b1_e = _tile_b(b1 + be2 @ W1)
    b2_e = _tile_b(b2)

    shared = dict(
        wq=wq, wk=wk, wv=wv, wo=wo, w1=w1, w2=w2,
        bq=bq_e, bk=bk_e, bo=bo_e, b1=b1_e, b2=b2_e,
    )
    in_maps = []
    for c in range(N_CORES):
        b = c // (N_CORES // B)
        qoff = (c % (N_CORES // B)) * TQ
        x_rot = np.roll(hs[b], -qoff, axis=0)
        m = dict(shared)
        m["x"] = np.ascontiguousarray(x_rot)
        in_maps.append(m)
    return in_maps


def kernel(**inputs):
    from concourse.bass_utils import run_bass_kernel_spmd

    nc = _get_nc()
    in_maps = prep_inmaps(inputs)
    res = run_bass_kernel_spmd(nc, in_maps, core_ids=list(range(N_CORES)))
    out = np.empty((B, S, H), dtype=np.float32)
    for c in range(N_CORES):
        b = c // (N_CORES // B)
        qoff = (c % (N_CORES // B)) * TQ
        out[b, qoff:qoff + TQ] = res.results[c]["out"]
    return out
